# revision 5
# baseline (speedup 1.0000x reference)
"""Bass/Trainium2 kernel for nn_Attention_369367188096 (sparse_attention).

Reference computation (B=2, N=4096, IN_DIM=1024, DIM=1024, HEADS=8, d=128):
    qkv = x @ W_qkv ; split into q,k,v per head
    dots = (q @ k^T) * DIM**-0.5 ; masked on top-left [2048,2048] block
    attn = softmax(dots) ; out = attn @ v ; out @ W_out + b_out

Sharding across 8 NeuronCores: core i handles batch b=i//4 and heads
(2*(i%4), 2*(i%4)+1).  Each core computes a partial output
x[b]-rows x DIM using its two heads' slice of W_out (row-sharded);
the host sums 4 partials per batch and adds b_out.

All matmul operands are bf16 (PE runs bf16 at 1 cycle/row vs 4x for fp32);
accumulation is fp32 in PSUM.  Softmax uses no max-subtraction: scores are
|s| <~ 1.5 after the 1/32 scale, so exp is numerically safe, and masking is
an exact 0/1 multiply after exp (identical to exp(-inf)=0).
"""

import os
import sys

for _p in ("/opt/trn_rl_repo", "/root/.axon_site/_ro/trn_rl_repo"):
    if os.path.isdir(_p) and _p not in sys.path:
        sys.path.insert(0, _p)

from contextlib import ExitStack

import ml_dtypes
import numpy as np

import concourse.bass as bass
import concourse.bacc as bacc
import concourse.mybir as mybir
import concourse.tile as tile
from concourse.bass_utils import run_bass_kernel_spmd

BF16 = mybir.dt.bfloat16
F32 = mybir.dt.float32
P = 128          # partitions
IN_DIM = 1024    # model in dim
OUT_DIM = 1024   # model out dim
DH = 128         # head dim
NH = 2           # heads per core
FD = 512         # matmul moving free dim
N_FULL = 4096    # sequence length
MM_FULL = 2048   # masked block size
SCALE = 1024 ** -0.5
N_CORES = 8


def build_nc(n=N_FULL, mm=MM_FULL):
    """Build the per-core Bass program (SPMD: same program, per-core data)."""
    CI = IN_DIM // P          # 8 input-dim chunks
    JC = n // P               # key chunks (32)
    IG = n // FD              # query groups of 512 (8)
    MJ = mm // P              # masked key chunks (16)
    MG = mm // FD             # masked query groups (4)
    EXPF = mybir.ActivationFunctionType.Exp

    nc = bacc.Bacc("TRN2", target_bir_lowering=False, debug=False)
    xt_d = nc.dram_tensor("xt", [IN_DIM, n], BF16, kind="ExternalInput")
    wq_d = nc.dram_tensor("wq", [IN_DIM, NH * DH], BF16, kind="ExternalInput")
    wk_d = nc.dram_tensor("wk", [IN_DIM, NH * DH], BF16, kind="ExternalInput")
    wv_d = nc.dram_tensor("wv", [IN_DIM, NH * DH], BF16, kind="ExternalInput")
    wo_d = nc.dram_tensor("wo", [NH * DH, OUT_DIM], BF16, kind="ExternalInput")
    mk_d = nc.dram_tensor("maskt", [mm, mm], BF16, kind="ExternalInput")
    out_d = nc.dram_tensor("part", [n, OUT_DIM], F32, kind="ExternalOutput")

    xt_v = xt_d.rearrange("(c p) n -> c p n", p=P)
    mk_v = mk_d.rearrange("(j p) i -> j p i", p=P)
    out_v = out_d.rearrange("(t p) o -> t p o", p=P)

    with tile.TileContext(nc) as tc, ExitStack() as ctx:
        const = ctx.enter_context(tc.tile_pool(name="const", bufs=1))

        # Resident inputs
        xt = [const.tile([P, n], BF16, tag=f"xt{c}", name=f"xt{c}") for c in range(CI)]
        for c in range(CI):
            nc.sync.dma_start(xt[c][:], xt_v[c])
        wq = const.tile([P, CI, NH * DH], BF16, tag="wq")
        wk = const.tile([P, CI, NH * DH], BF16, tag="wk")
        wv = const.tile([P, CI, NH * DH], BF16, tag="wv")
        for t, d in ((wq, wq_d), (wk, wk_d), (wv, wv_d)):
            nc.sync.dma_start(t[:], d.rearrange("(c p) d -> p c d", p=P))
        wo = const.tile([P, NH, OUT_DIM], BF16, tag="wo")
        nc.sync.dma_start(wo[:], wo_d.rearrange("(h p) o -> p h o", p=P))
        ones = const.tile([P, P], BF16, tag="ones")
        nc.vector.memset(ones[:], 1.0)

        # Resident intermediates
        qt = [const.tile([P, n], BF16, tag=f"qt{h}", name=f"qt{h}") for h in range(NH)]  # [d, i]
        kt = [const.tile([P, n], BF16, tag=f"kt{h}", name=f"kt{h}") for h in range(NH)]  # [d, j]
        vb = const.tile([P, JC, NH * DH], BF16, tag="vb")                 # [j, jc, (h d)]
        ot = [const.tile([P, n], BF16, tag=f"ot{h}", name=f"ot{h}") for h in range(NH)]  # [d, i]

        # ---- Phase 1: projections ----
        # Q^T, K^T per head: accumulate W[c,h].T @ x^T[c] over c.
        with tc.tile_pool(name="pq", bufs=4, space="PSUM") as pq:
            for h in range(NH):
                for w_sb, dst in ((wq, qt[h]), (wk, kt[h])):
                    for g0 in range(0, IG, 4):
                        gg = range(g0, min(g0 + 4, IG))
                        ps = [pq.tile([P, FD], F32, tag="pq", name="psqk") for _ in gg]
                        for c in range(CI):
                            for gi, g in enumerate(gg):
                                nc.tensor.matmul(
                                    ps[gi][:],
                                    w_sb[:, c, h * DH:(h + 1) * DH],
                                    xt[c][:, g * FD:(g + 1) * FD],
                                    start=(c == 0), stop=(c == CI - 1),
                                )
                        for gi, g in enumerate(gg):
                            nc.any.tensor_copy(dst[:, g * FD:(g + 1) * FD], ps[gi][:])
            # V (both heads) in natural [seq, d] layout: x^T[c] as weights.
            for t in range(JC):
                ps = pq.tile([P, NH * DH], F32, tag="pv")
                for c in range(CI):
                    nc.tensor.matmul(
                        ps[:], xt[c][:, t * P:(t + 1) * P], wv[:, c, :],
                        start=(c == 0), stop=(c == CI - 1),
                    )
                nc.any.tensor_copy(vb[:, t, :], ps[:])

        # ---- Phase 2: attention per head ----
        with (
            tc.tile_pool(name="pst", bufs=3, space="PSUM") as pst,
            tc.tile_pool(name="po", bufs=2, space="PSUM") as po,
            tc.tile_pool(name="pd", bufs=2, space="PSUM") as pd,
            tc.tile_pool(name="att", bufs=4) as att,
            tc.tile_pool(name="mkp", bufs=4) as mkp,
        ):
            for h in range(NH):
                for g in range(IG):
                    gs = g * FD
                    oacc = po.tile([P, FD], F32, tag="po")   # [d, i] accum
                    dacc = pd.tile([P, FD], F32, tag="pd")   # bcast denom accum
                    for j in range(JC):
                        st = pst.tile([P, FD], F32, tag="st")
                        nc.tensor.matmul(
                            st[:], kt[h][:, j * P:(j + 1) * P], qt[h][:, gs:gs + FD],
                            start=True, stop=True,
                        )
                        pt = att.tile([P, FD], BF16, tag="pt")
                        nc.scalar.activation(pt[:], st[:], EXPF, scale=SCALE)
                        if j < MJ and g < MG:
                            mt = mkp.tile([P, FD], BF16, tag="mt")
                            nc.sync.dma_start(mt[:], mk_v[j, :, gs:gs + FD])
                            nc.vector.tensor_mul(out=pt[:], in0=pt[:], in1=mt[:])
                        nc.tensor.matmul(
                            oacc[:], vb[:, j, h * DH:(h + 1) * DH], pt[:],
                            start=(j == 0), stop=(j == JC - 1),
                        )
                        nc.tensor.matmul(
                            dacc[:], ones[:], pt[:],
                            start=(j == 0), stop=(j == JC - 1),
                        )
                    rec = att.tile([P, FD], F32, tag="rec")
                    nc.vector.reciprocal(rec[:], dacc[:])
                    nc.vector.tensor_mul(
                        out=ot[h][:, gs:gs + FD], in0=oacc[:], in1=rec[:],
                    )

        # ---- Phase 3: output projection (partial over this core's heads) ----
        with (
            tc.tile_pool(name="pop", bufs=2, space="PSUM") as pop,
            tc.tile_pool(name="osp", bufs=3) as osp,
        ):
            for t in range(JC):
                pso = pop.tile([P, OUT_DIM], F32, tag="pop")
                for h in range(NH):
                    for nf in range(OUT_DIM // FD):
                        nc.tensor.matmul(
                            pso[:, nf * FD:(nf + 1) * FD],
                            ot[h][:, t * P:(t + 1) * P],
                            wo[:, h, nf * FD:(nf + 1) * FD],
                            start=(h == 0), stop=(h == NH - 1),
                        )
                ob = osp.tile([P, OUT_DIM], F32, tag="ob")
                nc.any.tensor_copy(ob[:], pso[:])
                nc.sync.dma_start(out_v[t], ob[:])

    nc.compile()
    return nc


def make_core_inputs(x, W_qkv, W_out, mask, n=N_FULL, mm=MM_FULL):
    """Host-side shard prep: per-core input dicts (bf16, pre-transposed)."""
    bf = ml_dtypes.bfloat16
    B = x.shape[0]
    xt_b = [np.ascontiguousarray(x[b].T).astype(bf) for b in range(B)]
    maskt = np.ascontiguousarray(mask[0, 0, :mm, :mm].T).astype(bf)
    cores_per_b = N_CORES // B
    in_maps = []
    for core in range(N_CORES):
        b = core // cores_per_b
        h0 = NH * (core % cores_per_b)
        qs, ks, vs = (W_qkv[:, o + h0 * DH: o + (h0 + NH) * DH]
                      for o in (0, OUT_DIM, 2 * OUT_DIM))
        in_maps.append({
            "xt": xt_b[b],
            "wq": np.ascontiguousarray(qs).astype(bf),
            "wk": np.ascontiguousarray(ks).astype(bf),
            "wv": np.ascontiguousarray(vs).astype(bf),
            "wo": np.ascontiguousarray(W_out[h0 * DH:(h0 + NH) * DH, :]).astype(bf),
            "maskt": maskt,
        })
    return in_maps


_NC_CACHE = {}


def _get_nc(n=N_FULL, mm=MM_FULL):
    key = (n, mm)
    if key not in _NC_CACHE:
        _NC_CACHE[key] = build_nc(n, mm)
    return _NC_CACHE[key]


def run(x, W_qkv, W_out, b_out, mask, trace=False, **trace_kwargs):
    nc = _get_nc()
    in_maps = make_core_inputs(x, W_qkv, W_out, mask)
    res = run_bass_kernel_spmd(
        nc, in_maps, list(range(N_CORES)), trace=trace, **trace_kwargs
    )
    B = x.shape[0]
    cores_per_b = N_CORES // B
    out = np.zeros((B, N_FULL, OUT_DIM), np.float32)
    for core in range(N_CORES):
        out[core // cores_per_b] += res.results[core]["part"]
    out += np.asarray(b_out, np.float32)
    return out, res


def kernel(x, W_qkv, W_out, b_out, mask, max_mask=MM_FULL, **_ignored):
    x = np.asarray(x, np.float32)
    W_qkv = np.asarray(W_qkv, np.float32)
    W_out = np.asarray(W_out, np.float32)
    b_out = np.asarray(b_out, np.float32)
    mask = np.asarray(mask)
    out, _ = run(x, W_qkv, W_out, b_out, mask)
    return out
